# revision 13
# baseline (speedup 1.0000x reference)
"""Trainium2 Bass kernel for nn_BinaryBlock (binary 3x3 conv block).

Reference semantics (forward values only):
    z   = prelu(x + bias1) + bias2          (per-channel prelu slope a)
    act = sign(z)                           (binary activation, +-1)
    bw  = sf[o] * sign(w)                   (sf = per-out-channel mean|w|)
    y   = conv3x3(act, bw, pad=1)
        + grouped_pool(x)                   (out o: pw[o,0]*x[2o]+pw[o,1]*x[2o+1])
    y   = pixel_unshuffle(y, 2)             (B,64,128,128) -> (B,256,64,64)

Kernel strategy (8 NeuronCores, data-parallel over batch, 2 images/core):
  * x is pre-cast to fp16 on host (halves HBM traffic; sign() and the fp16
    shortcut matmul are insensitive to the cast), output is written fp16 and
    upcast on host (values <= ~1.5, fp16 rel err 2^-11 << tolerance).
  * prelu chain is monotonic so act = sign(x + thr[c]) via ScalarE, writing
    fp8 +-1 into a padded per-band tile.
  * conv runs as fp8 DoubleRow matmuls with M=128: each stream processes
    row-pairs (a, a+1) of act; PE columns 0:64 compute out row a
    (weights [0, w(+1)]), columns 64:128 compute out row a+1
    ([w(-1), w(0)]).  Row parity selects the PSUM partition half.  Per
    8-row group and dj this needs only 2 matmuls (E: pairs starting at even
    rows, O: odd), each streaming the max 1024 fp8 values -> 512 cycles,
    double the per-cycle output of an M=64 scheme.
  * the grouped 1x1 shortcut (weights pw/sf) accumulates into the E bank via
    two concurrent col-tiled fp16 M=64 matmuls (even rows -> partitions
    0:64, odd -> 64:128).
  * combine: out row r = E + O contributions live at the same partition in
    two PSUM banks; VectorE adds them, writing fp16 directly in
    pixel-unshuffled layout (partition = (row-parity i, out chan o)), then
    one tensor_scalar multiplies by sf[o].  One DMA per (band, img) stores
    the band.
"""

import sys

import numpy as np

try:
    import concourse.bass as bass  # noqa: F401
except ImportError:  # pragma: no cover
    sys.path.insert(0, "/opt/trn_rl_repo")
    import concourse.bass as bass

import concourse.mybir as mybir
from concourse import bacc
from concourse.bass_utils import run_bass_kernel_spmd
from concourse.tile import TileContext

N_CORES = 8
B, C, H, W = 16, 128, 128, 128
CO = C // 2
BPC = B // N_CORES  # images per core
BAND = 32  # rows per band
NBAND = H // BAND
GR = 8  # output rows per group
NG = BAND // GR  # groups per band
AW = 144  # act row stride (>= W+2, multiple of 16 for DoubleRow Ko step)

_nc_cache = None


def _pair_rhs(act, row0, dj, npairs):
    """DoubleRow moving operand [K=128, Ko=2, pairs, cols=W]: Ko steps one
    act row; the pair dim steps two act rows."""
    base = act[:, row0 : row0 + 2 * npairs, dj : dj + W]
    ap = [list(d) for d in base.ap]
    ap[1] = [2 * AW, npairs]
    ap.insert(1, [AW, 2])
    return bass.AP(base.tensor, base.offset, ap)


def _strided_rows(xs, row0, nrows, stride=2):
    """[K=128, nrows (stride rows apart), W] view of the x tile."""
    base = xs[:, row0 : row0 + stride * nrows, :]
    ap = [list(d) for d in base.ap]
    ap[1] = [stride * W, nrows]
    return bass.AP(base.tensor, base.offset, ap)


def _slab_view(ps, h, s0, ns):
    """Read view of psum bank: [64p at half h, slabs s0:s0+ns, wo=64, j=2]."""
    base = ps[h * CO : (h + 1) * CO, s0 : s0 + ns, :]
    ap = [list(d) for d in base.ap]
    ap[-1:] = [[2, W // 2], [1, 2]]
    return bass.AP(base.tensor, base.offset, ap)


def _osb_view(osb, h, ho0, ns):
    """Matching write view of the band out tile [128=(i,o), j=2, ho=16, wo]:
    dims ordered (slab->ho, wo, j) to match _slab_view's read order."""
    base = osb[h * CO : (h + 1) * CO, :, ho0 : ho0 + ns, :]
    ap = [list(d) for d in base.ap]
    ap[1:] = [[W // 2, ns], [1, W // 2], [16 * (W // 2), 2]]
    return bass.AP(base.tensor, base.offset, ap)


def build_nc():
    f32 = mybir.dt.float32
    f16 = mybir.dt.float16
    fp8 = mybir.dt.float8e4
    DR = mybir.MatmulPerfMode.DoubleRow

    nc = bacc.Bacc()
    x_d = nc.dram_tensor("x", [BPC, C, H, W], f16, kind="ExternalInput")
    wte_d = nc.dram_tensor("wte", [C, 3, 2, C], fp8, kind="ExternalInput")
    wto_d = nc.dram_tensor("wto", [C, 3, 2, C], fp8, kind="ExternalInput")
    pw_d = nc.dram_tensor("pw", [C, CO], f16, kind="ExternalInput")
    thr_d = nc.dram_tensor("thr", [C, 1], f32, kind="ExternalInput")
    sf_d = nc.dram_tensor("sf", [C, 1], f32, kind="ExternalInput")
    y_d = nc.dram_tensor("y", [BPC, 4 * CO, H // 2, W // 2], f16, kind="ExternalOutput")
    # [img, o, f=(i j), ho, wo] with channel = 4o + f; one DMA per parity i
    y_v = y_d.rearrange("b (o f) h w -> b o f h w", f=4)

    with TileContext(nc) as tc:
        with (
            tc.tile_pool(name="cpool", bufs=1) as cpool,
            tc.tile_pool(name="xpool", bufs=3) as xpool,
            tc.tile_pool(name="apool", bufs=3) as apool,
            tc.tile_pool(name="opool", bufs=3) as opool,
            tc.tile_pool(name="pspool", bufs=4, space="PSUM") as pspool,
        ):
            wte = cpool.tile([C, 3, 2, C], fp8)
            nc.sync.dma_start(out=wte, in_=wte_d[:, :, :, :])
            wto = cpool.tile([C, 3, 2, C], fp8)
            nc.sync.dma_start(out=wto, in_=wto_d[:, :, :, :])
            pw = cpool.tile([C, CO], f16)
            nc.sync.dma_start(out=pw, in_=pw_d[:, :])
            thr = cpool.tile([C, 1], f32)
            nc.sync.dma_start(out=thr, in_=thr_d[:, :])
            sfv = cpool.tile([C, 1], f32)
            nc.sync.dma_start(out=sfv, in_=sf_d[:, :])

            # pending combine state: (E, tmp, osb, band, img, g) awaiting the
            # next group's scaled-O tile (last odd row's A-half contribution).
            # DVE reads at most one PSUM operand per op, so each O bank is
            # first folded to SBUF as tmp = O*sf (tensor_scalar, one PSUM in),
            # then combine does out = E*sf + tmp (scalar_tensor_tensor, one
            # PSUM in) -- which also absorbs the final sf scaling.
            pending = [None]
            MULT = mybir.AluOpType.mult
            ADD = mybir.AluOpType.add

            def _stt(out, h, E_view, tmp_view):
                nc.vector.scalar_tensor_tensor(
                    out=out,
                    in0=E_view,
                    scalar=sfv[h * CO : (h + 1) * CO, 0:1],
                    in1=tmp_view,
                    op0=MULT,
                    op1=ADD,
                )

            def combine(E, tmp, osb, g, tmp_next):
                ho0 = NG * g  # band-local ho of the group's first row pair
                # even rows (h0): E slabs 0:4 + tmp slabs 0:4
                _stt(
                    _osb_view(osb, 0, ho0, NG),
                    0,
                    _slab_view(E, 0, 0, NG),
                    _slab_view(tmp, 0, 0, NG),
                )
                # odd rows R+1,R+3,R+5 (h1): E slabs 0:3 + tmp slabs 1:4
                _stt(
                    _osb_view(osb, 1, ho0, NG - 1),
                    1,
                    _slab_view(E, 1, 0, NG - 1),
                    _slab_view(tmp, 1, 1, NG - 1),
                )
                # last odd row R+7: E slab 3 + next group's tmp slab 0
                _stt(
                    _osb_view(osb, 1, ho0 + NG - 1, 1),
                    1,
                    _slab_view(E, 1, NG - 1, 1),
                    _slab_view(tmp_next, 1, 0, 1),
                )

            def flush_pending(tmp_next):
                pE, ptmp, posb, pband, pimg, pg = pending[0]
                combine(pE, ptmp, posb, pg, tmp_next)
                if pg == NG - 1:
                    # band's osb complete: store
                    hos = slice(pband * (BAND // 2), (pband + 1) * (BAND // 2))
                    for i in range(2):
                        nc.sync.dma_start(
                            out=y_v[pimg, :, 2 * i : 2 * i + 2, hos, :],
                            in_=posb[i * CO : (i + 1) * CO, :, :, :],
                        )
                pending[0] = None

            for img in range(BPC):
                for band in range(NBAND):
                    r0 = band * BAND
                    lo = max(r0 - 1, 0)
                    hi = min(r0 + BAND + 1, H)
                    nrows = hi - lo
                    off = r0 - lo  # x tile row of output row r0
                    xs = xpool.tile([C, BAND + 2, W], f16, tag="xs", name=f"xs_{band}_{img}")
                    nc.sync.dma_start(out=xs[:, :nrows, :], in_=x_d[img, :, lo:hi, :])
                    act = apool.tile([C, BAND + 2, AW], fp8, tag="act", name=f"act_{band}_{img}")
                    nc.vector.memset(act[:, :, 0:1], 0.0)
                    nc.vector.memset(act[:, :, W + 1 : W + 2], 0.0)
                    row0 = 0
                    if band == 0:
                        nc.vector.memset(act[:, 0:1, : W + 2], 0.0)
                        row0 = 1
                    if band == NBAND - 1:
                        nc.vector.memset(act[:, BAND + 1 : BAND + 2, : W + 2], 0.0)
                    nc.scalar.sign(
                        act[:, row0 : row0 + nrows, 1 : W + 1],
                        xs[:, :nrows, :],
                        bias=thr[:, 0:1],
                    )
                    osb = opool.tile(
                        [C, 2, BAND // 2, W // 2], f16, tag="osb", name=f"osb_{band}_{img}"
                    )
                    for g in range(NG):
                        R = GR * g  # band-local first output row of the group
                        E = pspool.tile([C, NG, W], f32, tag="psE", name=f"psE_{band}_{g}_{img}")
                        O = pspool.tile([C, NG, W], f32, tag="psO", name=f"psO_{band}_{g}_{img}")
                        for dj in range(3):
                            # E pairs: act rows (R+2j, R+2j+1) -> tile rows +1
                            nc.tensor.matmul(
                                E[:, :, :],
                                wte[:, dj, :, :],
                                _pair_rhs(act, R + 1, dj, NG),
                                start=dj == 0,
                                stop=False,
                                perf_mode=DR,
                            )
                            # O pairs: act rows (R-1+2j, R+2j) -> tile rows +0
                            nc.tensor.matmul(
                                O[:, :, :],
                                wto[:, dj, :, :],
                                _pair_rhs(act, R, dj, NG),
                                start=dj == 0,
                                stop=dj == 2,
                                perf_mode=DR,
                            )
                        # shortcut: even rows into E[0:64], odd into E[64:128]
                        nc.tensor.matmul(
                            E[0:CO, :, :],
                            pw[:, :],
                            _strided_rows(xs, off + R, NG),
                            start=False,
                            stop=True,
                            skip_group_check=True,
                        )
                        nc.tensor.matmul(
                            E[CO:, :, :],
                            pw[:, :],
                            _strided_rows(xs, off + R + 1, NG),
                            start=False,
                            stop=True,
                            skip_group_check=True,
                        )
                        tmp = opool.tile(
                            [C, NG, W], f16, tag="otmp", name=f"tmp_{band}_{g}_{img}"
                        )
                        nc.vector.tensor_scalar_mul(
                            out=tmp, in0=O, scalar1=sfv[:, 0:1]
                        )
                        if pending[0] is not None:
                            flush_pending(tmp)
                        pending[0] = (E, tmp, osb, band, img, g)
                    act_last = act  # survives to the img-end flush
                # flush the img's last group: extra 1-pair O matmul for
                # stream S_127 (act rows 127, 128-pad -> last band tile
                # rows 32, 33); its h1 half is out row 127's (zero) bottom
                # halo tap.
                Ox = pspool.tile([C, 1, W], f32, tag="psO", name=f"psOx_{img}")
                for dj in range(3):
                    nc.tensor.matmul(
                        Ox[:, :, :],
                        wto[:, dj, :, :],
                        _pair_rhs(act_last, BAND, dj, 1),
                        start=dj == 0,
                        stop=dj == 2,
                        perf_mode=DR,
                    )
                tmp_x = opool.tile([C, 1, W], f16, tag="otmp", name=f"tmpx_{img}")
                nc.vector.tensor_scalar_mul(out=tmp_x, in0=Ox, scalar1=sfv[:, 0:1])
                flush_pending(tmp_x)
    nc.finalize()
    return nc


def prep_params(bias1, prelu_a, bias2, conv_w, pool_w):
    """Host-side folding of the tiny parameter tensors."""
    fp8np = mybir.dt.np(mybir.dt.float8e4)
    b1 = np.asarray(bias1, np.float64).reshape(C)
    a = np.asarray(prelu_a, np.float64).reshape(C)
    b2 = np.asarray(bias2, np.float64).reshape(C)
    if not np.all(a > 0):
        raise NotImplementedError("kernel assumes strictly positive PReLU slope")
    u0 = np.where(-b2 >= 0, -b2, -b2 / a)
    t = u0 - b1
    thr = (-t).astype(np.float32).reshape(C, 1)  # sign(x + thr) == sign(z)

    w = np.asarray(conv_w, np.float32).reshape(CO, C, 3, 3)
    sf = np.mean(np.abs(w), axis=(1, 2, 3), dtype=np.float32)  # [CO]
    wsign = np.sign(w).astype(np.float32)  # [CO, C, kh, kw]; kh = di+1
    # wte[k, dj, ko, m]: m = h*64+o
    #   h=0 (out row = pair row a): ko0 = 0, ko1 = w(di=+1)
    #   h=1 (out row a+1):          ko0 = w(di=-1), ko1 = w(di=0)
    wte = np.zeros((C, 3, 2, C), np.float32)
    wto = np.zeros((C, 3, 2, C), np.float32)
    for dj in range(3):
        wp1 = wsign[:, :, 2, dj].T  # [C(k), CO]  di=+1
        wm1 = wsign[:, :, 0, dj].T  # di=-1
        w0 = wsign[:, :, 1, dj].T  # di=0
        wte[:, dj, 1, 0:CO] = wp1
        wte[:, dj, 0, CO:] = wm1
        wte[:, dj, 1, CO:] = w0
        # O streams: pair row a odd -> h1 = row a (A-half), h0 = row a+1 (B)
        wto[:, dj, 1, CO:] = wp1
        wto[:, dj, 0, 0:CO] = wm1
        wto[:, dj, 1, 0:CO] = w0
    wte = wte.astype(fp8np)
    wto = wto.astype(fp8np)

    pwf = np.asarray(pool_w, np.float64).reshape(CO, 2)
    pwm = np.zeros((C, CO), np.float64)
    o = np.arange(CO)
    pwm[2 * o, o] = pwf[:, 0] / sf
    pwm[2 * o + 1, o] = pwf[:, 1] / sf
    pw16 = pwm.astype(np.float16)

    sfd = np.concatenate([sf, sf]).astype(np.float32).reshape(C, 1)
    return thr, wte, wto, pw16, sfd


def make_in_maps(x, bias1, prelu_a, bias2, conv_w, pool_w):
    thr, wte, wto, pw16, sfd = prep_params(bias1, prelu_a, bias2, conv_w, pool_w)
    x = np.ascontiguousarray(np.asarray(x, np.float16))
    assert x.shape == (B, C, H, W), x.shape
    return [
        {
            "x": x[i * BPC : (i + 1) * BPC],
            "wte": wte,
            "wto": wto,
            "pw": pw16,
            "thr": thr,
            "sf": sfd,
        }
        for i in range(N_CORES)
    ]


def kernel(x, bias1, prelu_a, bias2, conv_w, pool_w):
    global _nc_cache
    in_maps = make_in_maps(x, bias1, prelu_a, bias2, conv_w, pool_w)
    if _nc_cache is None:
        _nc_cache = build_nc()
    res = run_bass_kernel_spmd(_nc_cache, in_maps, list(range(N_CORES)))
    y = np.concatenate([res.results[i]["y"] for i in range(N_CORES)], axis=0)
    return np.ascontiguousarray(y.astype(np.float32))


# revision 16
# speedup vs baseline: 1.6054x; 1.6054x over previous
"""Trainium2 Bass kernel for nn_BinaryBlock (binary 3x3 conv block).

Reference semantics (forward values only):
    z   = prelu(x + bias1) + bias2          (per-channel prelu slope a)
    act = sign(z)                           (binary activation, +-1)
    bw  = sf[o] * sign(w)                   (sf = per-out-channel mean|w|)
    y   = conv3x3(act, bw, pad=1)
        + grouped_pool(x)                   (out o: pw[o,0]*x[2o]+pw[o,1]*x[2o+1])
    y   = pixel_unshuffle(y, 2)             (B,64,128,128) -> (B,256,64,64)

Kernel strategy (8 NeuronCores, data-parallel over batch, 2 images/core):
  * x is pre-cast to fp16 on host (halves HBM traffic; sign() and the fp16
    shortcut matmul are insensitive to the cast), output is written fp16 and
    upcast on host (values <= ~1.5, fp16 rel err 2^-11 << tolerance).
  * prelu chain is monotonic so act = sign(x + thr[c]) via ScalarE, writing
    fp8 +-1 into a padded per-band tile.
  * conv runs as fp8 DoubleRow matmuls with M=128: each stream processes
    row-pairs (a, a+1) of act; PE columns 0:64 compute out row a with
    weights [0, w(+1)], columns 64:128 compute out row a+1 with
    [w(-1), w(0)].  Row parity = PSUM partition half.  Per 8-row group and
    kernel column dj this needs only 2 matmuls (E: pairs starting at even
    rows, O: odd), each streaming the max 1024 fp8 values -> 512 cycles,
    using the whole PE array.  Out row r = E-half + O-half, summed later.
  * matmul PSUM out APs walk (slab, wo, j) so PSUM lands pixel-unshuffled
    ([j][slab][wo]); every downstream op is a contiguous 2/3D slice and the
    output DMA does the final (j, ho) -> channel reorder for free.
  * the grouped 1x1 shortcut (weights pw/sf) accumulates into the E bank via
    two concurrent col-tiled fp16 M=64 matmuls (even rows -> partitions
    0:64, odd -> 64:128).
  * combine: ScalarE folds each O bank to SBUF fp16 as tmp = O*sf (one PSUM
    read); VectorE then does osb = E*sf + tmp via scalar_tensor_tensor (one
    PSUM read), absorbing the sf scaling.  One DMA per (band, img, parity)
    stores the band.
"""

import sys

import numpy as np

try:
    import concourse.bass as bass  # noqa: F401
except ImportError:  # pragma: no cover
    sys.path.insert(0, "/opt/trn_rl_repo")
    import concourse.bass as bass

import concourse.mybir as mybir
from concourse import bacc
from concourse.bass_utils import run_bass_kernel_spmd
from concourse.tile import TileContext

N_CORES = 8
B, C, H, W = 16, 128, 128, 128
CO = C // 2
BPC = B // N_CORES  # images per core
BAND = 32  # rows per band
NBAND = H // BAND
GR = 8  # output rows per group
NG = BAND // GR  # groups per band
WO = W // 2
AW = 144  # act row stride (>= W+2, multiple of 16 for DoubleRow Ko step)

_nc_cache = None


def _pair_rhs(act, row0, dj, npairs):
    """DoubleRow moving operand [K=128, Ko=2, pairs, cols=W]: Ko steps one
    act row; the pair dim steps two act rows."""
    base = act[:, row0 : row0 + 2 * npairs, dj : dj + W]
    ap = [list(d) for d in base.ap]
    ap[1] = [2 * AW, npairs]
    ap.insert(1, [AW, 2])
    return bass.AP(base.tensor, base.offset, ap)


def _strided_rows(xs, row0, nrows, stride=2):
    """[K=128, nrows (stride rows apart), W] view of the x tile."""
    base = xs[:, row0 : row0 + stride * nrows, :]
    ap = [list(d) for d in base.ap]
    ap[1] = [stride * W, nrows]
    return bass.AP(base.tensor, base.offset, ap)


def _mm_out(ps, h0, nh):
    """Matmul out AP over psum tile [128, 2, nslab, WO]: stream column
    c = 2*wo + j of slab s lands at [j][s][wo] (walk order slab, wo, j)."""
    nslab = ps.shape[2]
    base = ps[h0 : h0 + nh, :, :, :]
    ap = [list(base.ap[0]), [WO, nslab], [1, WO], [nslab * WO, 2]]
    return bass.AP(base.tensor, base.offset, ap)


def build_nc():
    f32 = mybir.dt.float32
    f16 = mybir.dt.float16
    fp8 = mybir.dt.float8e4
    DR = mybir.MatmulPerfMode.DoubleRow

    nc = bacc.Bacc()
    x_d = nc.dram_tensor("x", [BPC, C, H, W], f16, kind="ExternalInput")
    wte_d = nc.dram_tensor("wte", [C, 3, 2, C], fp8, kind="ExternalInput")
    wto_d = nc.dram_tensor("wto", [C, 3, 2, C], fp8, kind="ExternalInput")
    pw_d = nc.dram_tensor("pw", [C, CO], f16, kind="ExternalInput")
    thr_d = nc.dram_tensor("thr", [C, 1], f32, kind="ExternalInput")
    sf_d = nc.dram_tensor("sf", [C, 1], f32, kind="ExternalInput")
    y_d = nc.dram_tensor("y", [BPC, 4 * CO, H // 2, WO], f16, kind="ExternalOutput")
    # [img, o, f=(i j), ho, wo]: channel = 4o + f; for fixed parity i the
    # DMA iterates (j, ho, wo) and (ho, wo) merges into one contiguous run
    y_v = y_d.rearrange("b (o f) h w -> b o f h w", f=4)

    with TileContext(nc) as tc:
        with (
            tc.tile_pool(name="cpool", bufs=1) as cpool,
            tc.tile_pool(name="xpool", bufs=3) as xpool,
            tc.tile_pool(name="apool", bufs=3) as apool,
            tc.tile_pool(name="opool", bufs=3) as opool,
            tc.tile_pool(name="pspool", bufs=4, space="PSUM") as pspool,
        ):
            wte = cpool.tile([C, 3, 2, C], fp8)
            nc.sync.dma_start(out=wte, in_=wte_d[:, :, :, :])
            wto = cpool.tile([C, 3, 2, C], fp8)
            nc.sync.dma_start(out=wto, in_=wto_d[:, :, :, :])
            pw = cpool.tile([C, CO], f16)
            nc.sync.dma_start(out=pw, in_=pw_d[:, :])
            thr = cpool.tile([C, 1], f32)
            nc.sync.dma_start(out=thr, in_=thr_d[:, :])
            sfv = cpool.tile([C, 1], f32)
            nc.sync.dma_start(out=sfv, in_=sf_d[:, :])

            # pending combine state: (E, tmp, osb, band, img, g) awaiting the
            # next group's scaled-O tile (last odd row's A-half contribution)
            pending = [None]
            MULT = mybir.AluOpType.mult
            ADD = mybir.AluOpType.add

            def _stt(out, h, E_view, tmp_view):
                nc.vector.scalar_tensor_tensor(
                    out=out,
                    in0=E_view,
                    scalar=sfv[h * CO : (h + 1) * CO, 0:1],
                    in1=tmp_view,
                    op0=MULT,
                    op1=ADD,
                )

            def combine(E, tmp, osb, g, tmp_next):
                ho0 = NG * g  # band-local ho of the group's first row pair
                # even rows (h0): E slabs 0:4 + tmp slabs 0:4
                _stt(
                    osb[0:CO, :, ho0 : ho0 + NG, :],
                    0,
                    E[0:CO, :, :, :],
                    tmp[0:CO, :, :, :],
                )
                # odd rows R+1,R+3,R+5 (h1): E slabs 0:3 + tmp slabs 1:4
                _stt(
                    osb[CO:, :, ho0 : ho0 + NG - 1, :],
                    1,
                    E[CO:, :, 0 : NG - 1, :],
                    tmp[CO:, :, 1:NG, :],
                )
                # last odd row R+7: E slab 3 + next group's tmp slab 0
                _stt(
                    osb[CO:, :, ho0 + NG - 1 : ho0 + NG, :],
                    1,
                    E[CO:, :, NG - 1 : NG, :],
                    tmp_next[CO:, :, 0:1, :],
                )

            def flush_pending(tmp_next):
                pE, ptmp, posb, pband, pimg, pg = pending[0]
                combine(pE, ptmp, posb, pg, tmp_next)
                if pg == NG - 1:
                    # band's osb complete: store one DMA per row parity
                    hos = slice(pband * (BAND // 2), (pband + 1) * (BAND // 2))
                    for i in range(2):
                        nc.sync.dma_start(
                            out=y_v[pimg, :, 2 * i : 2 * i + 2, hos, :],
                            in_=posb[i * CO : (i + 1) * CO, :, :, :],
                        )
                pending[0] = None

            for img in range(BPC):
                for band in range(NBAND):
                    r0 = band * BAND
                    lo = max(r0 - 1, 0)
                    hi = min(r0 + BAND + 1, H)
                    nrows = hi - lo
                    off = r0 - lo  # x tile row of output row r0
                    xs = xpool.tile(
                        [C, BAND + 2, W], f16, tag="xs", name=f"xs_{band}_{img}"
                    )
                    nc.sync.dma_start(out=xs[:, :nrows, :], in_=x_d[img, :, lo:hi, :])
                    act = apool.tile(
                        [C, BAND + 2, AW], fp8, tag="act", name=f"act_{band}_{img}"
                    )
                    nc.vector.memset(act[:, :, 0:1], 0.0)
                    nc.vector.memset(act[:, :, W + 1 : W + 2], 0.0)
                    row0 = 0
                    if band == 0:
                        nc.vector.memset(act[:, 0:1, : W + 2], 0.0)
                        row0 = 1
                    if band == NBAND - 1:
                        nc.vector.memset(act[:, BAND + 1 : BAND + 2, : W + 2], 0.0)
                    nc.scalar.sign(
                        act[:, row0 : row0 + nrows, 1 : W + 1],
                        xs[:, :nrows, :],
                        bias=thr[:, 0:1],
                    )
                    osb = opool.tile(
                        [C, 2, BAND // 2, WO], f16, tag="osb", name=f"osb_{band}_{img}"
                    )
                    for g in range(NG):
                        R = GR * g  # band-local first output row of the group
                        E = pspool.tile(
                            [C, 2, NG, WO], f32, tag="psE", name=f"psE_{band}_{g}_{img}"
                        )
                        O = pspool.tile(
                            [C, 2, NG, WO], f32, tag="psO", name=f"psO_{band}_{g}_{img}"
                        )
                        for dj in range(3):
                            # E pairs: act rows (R+2j, R+2j+1) -> tile rows +1
                            nc.tensor.matmul(
                                _mm_out(E, 0, C),
                                wte[:, dj, :, :],
                                _pair_rhs(act, R + 1, dj, NG),
                                start=dj == 0,
                                stop=False,
                                perf_mode=DR,
                            )
                            # O pairs: act rows (R-1+2j, R+2j) -> tile rows +0
                            nc.tensor.matmul(
                                _mm_out(O, 0, C),
                                wto[:, dj, :, :],
                                _pair_rhs(act, R, dj, NG),
                                start=dj == 0,
                                stop=dj == 2,
                                perf_mode=DR,
                            )
                        # shortcut: even rows into E[0:64], odd into E[64:128]
                        nc.tensor.matmul(
                            _mm_out(E, 0, CO),
                            pw[:, :],
                            _strided_rows(xs, off + R, NG),
                            start=False,
                            stop=True,
                            skip_group_check=True,
                        )
                        nc.tensor.matmul(
                            _mm_out(E, CO, CO),
                            pw[:, :],
                            _strided_rows(xs, off + R + 1, NG),
                            start=False,
                            stop=True,
                            skip_group_check=True,
                        )
                        # fold O to SBUF with the sf scale on ScalarE
                        tmp = opool.tile(
                            [C, 2, NG, WO], f16, tag="otmp", name=f"tmp_{band}_{g}_{img}"
                        )
                        nc.scalar.mul(tmp, O, sfv[:, 0:1])
                        if pending[0] is not None:
                            flush_pending(tmp)
                        pending[0] = (E, tmp, osb, band, img, g)
                    act_last = act  # survives to the img-end flush
                # flush the img's last group: extra 1-pair O matmul for
                # stream S_127 (act rows 127, 128-pad -> last band tile rows
                # 32, 33); its h1 half is out row 127's (zero) bottom halo tap.
                Ox = pspool.tile([C, 2, 1, WO], f32, tag="psO", name=f"psOx_{img}")
                for dj in range(3):
                    nc.tensor.matmul(
                        _mm_out(Ox, 0, C),
                        wto[:, dj, :, :],
                        _pair_rhs(act_last, BAND, dj, 1),
                        start=dj == 0,
                        stop=dj == 2,
                        perf_mode=DR,
                    )
                tmp_x = opool.tile([C, 2, 1, WO], f16, tag="otmp", name=f"tmpx_{img}")
                nc.scalar.mul(tmp_x, Ox, sfv[:, 0:1])
                flush_pending(tmp_x)
    nc.finalize()
    return nc


def prep_params(bias1, prelu_a, bias2, conv_w, pool_w):
    """Host-side folding of the tiny parameter tensors."""
    fp8np = mybir.dt.np(mybir.dt.float8e4)
    b1 = np.asarray(bias1, np.float64).reshape(C)
    a = np.asarray(prelu_a, np.float64).reshape(C)
    b2 = np.asarray(bias2, np.float64).reshape(C)
    if not np.all(a > 0):
        raise NotImplementedError("kernel assumes strictly positive PReLU slope")
    u0 = np.where(-b2 >= 0, -b2, -b2 / a)
    t = u0 - b1
    thr = (-t).astype(np.float32).reshape(C, 1)  # sign(x + thr) == sign(z)

    w = np.asarray(conv_w, np.float32).reshape(CO, C, 3, 3)
    sf = np.mean(np.abs(w), axis=(1, 2, 3), dtype=np.float32)  # [CO]
    wsign = np.sign(w).astype(np.float32)  # [CO, C, kh, kw]; kh = di+1
    # wte[k, dj, ko, m]: m = h*64+o
    #   h=0 (out row = pair row a): ko0 = 0, ko1 = w(di=+1)
    #   h=1 (out row a+1):          ko0 = w(di=-1), ko1 = w(di=0)
    # wto = same with halves swapped (odd pair rows -> h1 is the A-half)
    wte = np.zeros((C, 3, 2, C), np.float32)
    wto = np.zeros((C, 3, 2, C), np.float32)
    for dj in range(3):
        wp1 = wsign[:, :, 2, dj].T  # [C(k), CO]  di=+1
        wm1 = wsign[:, :, 0, dj].T  # di=-1
        w0 = wsign[:, :, 1, dj].T  # di=0
        wte[:, dj, 1, 0:CO] = wp1
        wte[:, dj, 0, CO:] = wm1
        wte[:, dj, 1, CO:] = w0
        wto[:, dj, 1, CO:] = wp1
        wto[:, dj, 0, 0:CO] = wm1
        wto[:, dj, 1, 0:CO] = w0
    wte = wte.astype(fp8np)
    wto = wto.astype(fp8np)

    pwf = np.asarray(pool_w, np.float64).reshape(CO, 2)
    pwm = np.zeros((C, CO), np.float64)
    o = np.arange(CO)
    pwm[2 * o, o] = pwf[:, 0] / sf
    pwm[2 * o + 1, o] = pwf[:, 1] / sf
    pw16 = pwm.astype(np.float16)

    sfd = np.concatenate([sf, sf]).astype(np.float32).reshape(C, 1)
    return thr, wte, wto, pw16, sfd


def make_in_maps(x, bias1, prelu_a, bias2, conv_w, pool_w):
    thr, wte, wto, pw16, sfd = prep_params(bias1, prelu_a, bias2, conv_w, pool_w)
    x = np.ascontiguousarray(np.asarray(x, np.float16))
    assert x.shape == (B, C, H, W), x.shape
    return [
        {
            "x": x[i * BPC : (i + 1) * BPC],
            "wte": wte,
            "wto": wto,
            "pw": pw16,
            "thr": thr,
            "sf": sfd,
        }
        for i in range(N_CORES)
    ]


def kernel(x, bias1, prelu_a, bias2, conv_w, pool_w):
    global _nc_cache
    in_maps = make_in_maps(x, bias1, prelu_a, bias2, conv_w, pool_w)
    if _nc_cache is None:
        _nc_cache = build_nc()
    res = run_bass_kernel_spmd(_nc_cache, in_maps, list(range(N_CORES)))
    y = np.concatenate([res.results[i]["y"] for i in range(N_CORES)], axis=0)
    return np.ascontiguousarray(y.astype(np.float32))
